# revision 3
# baseline (speedup 1.0000x reference)
"""Distributed Trainium2 kernel for the two-stage latent-attention module.

Strategy (8 NeuronCores, sequence-parallel over the node axis n):
  - Each core owns n-slice [i*1024, (i+1)*1024) of every batch.
  - Stage 1 (learned queries attend over n) is computed as unnormalized
    per-core partials (exp-weighted sums + denominators via an appended
    ones-column on v1); one AllReduce(add) of the tiny [65, B, H, M]
    partial combines them.
  - Stage 2 is fully local (rowwise in n); output shards are concatenated
    on the host.
  - All matmuls run as float32r (TF32-like, 1 cycle/row at free-dim>=256).
  - All biases are folded away: k-side biases drop by softmax shift
    invariance; v-side biases are constants after attention-weight
    normalization and are folded into an effective output bias on host.
"""

import sys

sys.path.insert(0, "/opt/trn_rl_repo")

import numpy as np

import concourse.bass as bass
import concourse.mybir as mybir
import concourse.tile as tile_mod
from bass_rust import ScopedClock
from concourse.bass_utils import run_bass_kernel_spmd
from concourse.masks import make_identity
from concourse.tile import TileContext

B, N, C, H, HC, M = 4, 8192, 512, 8, 64, 256
NCORES = 8
NLOC = N // NCORES          # 1024 tokens per batch per core
KO = C // 128               # 4 contraction tiles
F32 = mybir.dt.float32
F32R = mybir.dt.float32r
EXP = mybir.ActivationFunctionType.Exp
MAX_WAITS = 1               # this walrus rejects >1 sem wait per instruction

TRACE = False               # set by test harness for profiling runs


def _patch_tile_drain():
    """TileContext's exit drain carries one sem wait per live clock proc;
    walrus here allows only one wait per instruction, so emit the waits as
    a chain of standalone wait_ge instructions instead."""

    def _drain_and_barrier(self, tick_clock, wait_clock):
        nc = self.nc
        probe = nc.sync.nop(nofuse=True, hint="drain_wait_probe")
        wait_clock.add_sem_waits(probe.ins, ScopedClock({None: tick_clock.global_clock}))
        si = probe.ins.sync_info
        waits = list(si.on_wait) if si and si.on_wait else []
        if si and si.on_wait:
            si.on_wait.clear()
        sems = {h.num: h for h in self.sems.allocated().values()}
        for w in waits:
            assert w.wait_mode == "sem-ge-imm", w
            nc.sync.wait_ge(sems[w.id], w.wait_value)
        nc.sync.drain()
        nc.all_engine_barrier()
        popped = nc._tile_sem_poison_stack.pop()
        assert popped is self._sem_poison
        nc.clear_and_free_semaphores(list(self.sems.allocated().values()))
        nc.all_engine_barrier()

    tile_mod.TileContext._drain_and_barrier = _drain_and_barrier


def _split_excess_sync_waits(nc, max_waits=MAX_WAITS):
    """Hoist excess sem waits onto same-engine nops placed just before the
    over-constrained instruction (walrus limit: one sync wait each)."""
    ctr = 0
    for f in nc.m.functions:
        for bb in f.blocks:
            insts = list(bb.instructions)
            new_list = []
            changed = False
            for inst in insts:
                si = inst.sync_info
                if si is not None and si.on_wait and len(si.on_wait) > max_waits:
                    waits = list(si.on_wait)
                    movable = [w for w in waits
                               if w.sync_type == "semaphore" and w.wait_reg is None]
                    keep = [w for w in waits
                            if not (w.sync_type == "semaphore" and w.wait_reg is None)]
                    n_keep = max(0, max_waits - len(keep))
                    keep_sem = movable[:n_keep] if n_keep else []
                    excess = movable[len(keep_sem):]
                    assert len(keep) + len(keep_sem) <= max_waits, (inst.name, waits)
                    si.on_wait.clear()
                    for w in keep + keep_sem:
                        si.on_wait.append(w)
                    for i in range(0, len(excess), max_waits):
                        nop = mybir.InstNoOp(name=f"I-waitsplit-{ctr}", ins=[], outs=[])
                        ctr += 1
                        nop.engine = inst.engine
                        nop.sync_info = mybir.SyncInfo(on_wait=[], on_update=[])
                        for w in excess[i:i + max_waits]:
                            nop.sync_info.on_wait.append(w)
                        new_list.append(nop)
                    changed = True
                new_list.append(inst)
            if changed:
                bb.instructions = new_list
    return ctr


def _build():
    _patch_tile_drain()
    nc = bass.Bass("TRN2")

    xT_ext = nc.declare_dram_parameter("xT", [B, C, NLOC], F32, isOutput=False)
    qT_ext = nc.declare_dram_parameter("qT", [128, KO, M], F32, isOutput=False)
    wk1_ext = nc.declare_dram_parameter("wk1", [128, KO, C], F32, isOutput=False)
    wv1_ext = nc.declare_dram_parameter("wv1", [128, KO, C], F32, isOutput=False)
    wq3_ext = nc.declare_dram_parameter("wq3", [128, KO, C], F32, isOutput=False)
    wout_ext = nc.declare_dram_parameter("wout", [128, KO, C], F32, isOutput=False)
    wkv2e_ext = nc.declare_dram_parameter("wkv2e", [HC, 128], F32, isOutput=False)
    wkv2o_ext = nc.declare_dram_parameter("wkv2o", [HC, 128], F32, isOutput=False)
    bq3_ext = nc.declare_dram_parameter("bq3", [128, KO], F32, isOutput=False)
    bout_ext = nc.declare_dram_parameter("bout", [1, C], F32, isOutput=False)
    out_ext = nc.declare_dram_parameter("out", [B, NLOC, C], F32, isOutput=True)

    # DRAM scratch
    q3spill = nc.dram_tensor("q3spill", [B, 128, KO, NLOC], F32R)
    cc_in = [nc.dram_tensor(f"cc_in{i}", [65, 2, H, M], F32) for i in range(2)]
    cc_out = [nc.dram_tensor(f"cc_out{i}", [65, 2, H, M], F32, addr_space="Shared")
              for i in range(2)]
    groups = [list(range(NCORES))]

    with TileContext(nc) as tc:
        with tc.tile_pool(name="w", bufs=1) as wp, \
             tc.tile_pool(name="out1p", bufs=1) as o1p:
            wk1 = wp.tile([128, KO, C], F32R)
            wv1 = wp.tile([128, KO, C], F32R)
            wq3 = wp.tile([128, KO, C], F32R)
            wout = wp.tile([128, KO, C], F32R)
            wkv2e = wp.tile([HC, 128], F32R)
            wkv2o = wp.tile([HC, 128], F32R)
            qT = wp.tile([128, KO, M], F32R)
            bq3 = wp.tile([128, KO], F32)
            bout = wp.tile([1, C], F32R)
            for dst, src in [(wk1, wk1_ext), (wv1, wv1_ext), (wq3, wq3_ext),
                             (wout, wout_ext), (wkv2e, wkv2e_ext),
                             (wkv2o, wkv2o_ext), (qT, qT_ext), (bout, bout_ext)]:
                nc.gpsimd.dma_start(out=dst[:], in_=src[:])
            nc.sync.dma_start(out=bq3[:], in_=bq3_ext[:])
            ident_f = wp.tile([128, 128], F32)
            make_identity(nc, ident_f[:])
            ident = wp.tile([128, 128], F32R)
            nc.vector.tensor_copy(out=ident[:], in_=ident_f[:])
            ones_f = wp.tile([128, 2], F32)
            nc.vector.memset(ones_f[:], 1.0)
            ones64 = wp.tile([1, HC], F32R)
            nc.vector.tensor_copy(out=ones64[:], in_=ones_f[0:1, 0:1].to_broadcast([1, HC]))
            ones128 = wp.tile([1, 128], F32R)
            nc.vector.tensor_copy(out=ones128[:], in_=ones_f[0:1, 0:1].to_broadcast([1, 128]))
            onescol = wp.tile([128, 1], F32R)
            nc.vector.tensor_copy(out=onescol[:], in_=ones_f[:, 0:1])

            out1p_all = o1p.tile([65, B, H, M], F32)

            # ---------------- Phase A: stage 1 partials + q3 ----------------
            with tc.tile_pool(name="xT", bufs=2) as xp, \
                 tc.tile_pool(name="k1T", bufs=1) as k1p, \
                 tc.tile_pool(name="v1", bufs=1) as v1p, \
                 tc.tile_pool(name="q3T", bufs=2) as q3p, \
                 tc.tile_pool(name="s1", bufs=2) as s1p, \
                 tc.tile_pool(name="psA512", bufs=2, space="PSUM") as psA512, \
                 tc.tile_pool(name="psAlog", bufs=4, space="PSUM") as psAlog, \
                 tc.tile_pool(name="psAav1", bufs=2, space="PSUM") as psAav1:
                for b in range(B):
                    xT = xp.tile([128, KO, NLOC], F32R)
                    nc.gpsimd.dma_start(
                        out=xT[:], in_=xT_ext[b].rearrange("(ko p) t -> p ko t", p=128))

                    k1T = k1p.tile([128, KO, NLOC], F32R)
                    for fo in range(KO):
                        for t5 in range(2):
                            ps = psA512.tile([128, 512], F32, tag="psA512")
                            for ko in range(KO):
                                nc.tensor.matmul(
                                    ps[:], wk1[:, ko, fo * 128:(fo + 1) * 128],
                                    xT[:, ko, t5 * 512:(t5 + 1) * 512],
                                    start=(ko == 0), stop=(ko == KO - 1))
                            nc.vector.tensor_copy(
                                out=k1T[:, fo, t5 * 512:(t5 + 1) * 512], in_=ps[:])

                    v1aug = v1p.tile([128, 8, H, HC + 1], F32R)
                    nc.vector.tensor_copy(
                        out=v1aug[:, :, :, HC:HC + 1],
                        in_=onescol[:, 0:1].to_broadcast([128, 8, H, 1]))
                    for t8 in range(8):
                        ps = psA512.tile([128, 512], F32, tag="psA512")
                        for ko in range(KO):
                            nc.tensor.matmul(
                                ps[:], xT[:, ko, t8 * 128:(t8 + 1) * 128],
                                wv1[:, ko, :],
                                start=(ko == 0), stop=(ko == KO - 1))
                        nc.vector.tensor_copy(
                            out=v1aug[:, t8, :, 0:HC],
                            in_=ps[:].rearrange("p (h c) -> p h c", h=H))

                    q3T = q3p.tile([128, KO, NLOC], F32R, tag="q3T")
                    for fo in range(KO):
                        for t5 in range(2):
                            ps = psA512.tile([128, 512], F32, tag="psA512")
                            for ko in range(KO):
                                nc.tensor.matmul(
                                    ps[:], wq3[:, ko, fo * 128:(fo + 1) * 128],
                                    xT[:, ko, t5 * 512:(t5 + 1) * 512],
                                    start=(ko == 0), stop=(ko == KO - 1))
                            nc.vector.tensor_tensor(
                                out=q3T[:, fo, t5 * 512:(t5 + 1) * 512], in0=ps[:],
                                in1=bq3[:, fo:fo + 1].to_broadcast([128, 512]),
                                op=mybir.AluOpType.add)
                    nc.sync.dma_start(out=q3spill[b], in_=q3T[:])

                    for h in range(H):
                        po = (h % 2) * 64
                        s1 = s1p.tile([128, 8, M], F32R, tag="s1")
                        for t8 in range(8):
                            ps = psAlog.tile([128, M], F32, tag="psAlog")
                            nc.tensor.matmul(
                                ps[:],
                                k1T[po:po + 64, h // 2, t8 * 128:(t8 + 1) * 128],
                                qT[po:po + 64, h // 2, :],
                                start=True, stop=True)
                            nc.scalar.activation(s1[:, t8, :], ps[:], EXP, scale=0.125)
                        psv = psAav1.tile([65, M], F32, tag="psAav1")
                        for t8 in range(8):
                            nc.tensor.matmul(
                                psv[:], v1aug[:, t8, h, :], s1[:, t8, :],
                                start=(t8 == 0), stop=(t8 == 7))
                        nc.vector.tensor_copy(out=out1p_all[:, b, h, :], in_=psv[:])

                    if b == 1:
                        nc.sync.dma_start(out=cc_in[0][:], in_=out1p_all[:, 0:2])
                        nc.gpsimd.collective_compute(
                            "AllReduce", mybir.AluOpType.add, replica_groups=groups,
                            ins=[cc_in[0][:]], outs=[cc_out[0][:]])
                    if b == 3:
                        nc.sync.dma_start(out=cc_in[1][:], in_=out1p_all[:, 2:4])
                        nc.gpsimd.collective_compute(
                            "AllReduce", mybir.AluOpType.add, replica_groups=groups,
                            ins=[cc_in[1][:]], outs=[cc_out[1][:]])

            # ---------------- Phase C: stage 2 + output projection ----------
            with tc.tile_pool(name="g", bufs=1) as gp, \
                 tc.tile_pool(name="q3Tc", bufs=2) as q3pc, \
                 tc.tile_pool(name="k2sb", bufs=1) as k2p, \
                 tc.tile_pool(name="v2", bufs=2) as v2p, \
                 tc.tile_pool(name="p2", bufs=2) as p2p, \
                 tc.tile_pool(name="yT", bufs=1) as ytp, \
                 tc.tile_pool(name="osb", bufs=3) as op, \
                 tc.tile_pool(name="sm", bufs=2) as smp, \
                 tc.tile_pool(name="psC512", bufs=3, space="PSUM") as psC512, \
                 tc.tile_pool(name="psCav2", bufs=2, space="PSUM") as psCav2, \
                 tc.tile_pool(name="psCbc", bufs=1, space="PSUM") as psCbc, \
                 tc.tile_pool(name="psCkv2", bufs=1, space="PSUM") as psCkv2, \
                 tc.tile_pool(name="psCtr", bufs=1, space="PSUM") as psCtr:
                out1p_g = [gp.tile([65, 2, H, M], F32, tag=f"g{i}", name=f"g{i}")
                           for i in range(2)]
                nc.sync.dma_start(out=out1p_g[0][:], in_=cc_out[0][:])
                nc.sync.dma_start(out=out1p_g[1][:], in_=cc_out[1][:])

                for b in range(B):
                    g = out1p_g[b // 2]
                    bh = b % 2
                    q3T = q3pc.tile([128, KO, NLOC], F32R, tag="q3Tc")
                    nc.sync.dma_start(out=q3T[:], in_=q3spill[b])

                    yT = ytp.tile([128, KO, NLOC], F32R, tag="yT")
                    for hp in range(4):
                        k2sb = k2p.tile([128, M], F32R, tag=f"k2sb{hp}")
                        v2augs = []
                        for h in (2 * hp, 2 * hp + 1):
                            po = (h % 2) * 64
                            # normalize out1: out1T = out1p/d1 (rows 0..63)
                            dsb = smp.tile([1, M], F32R, tag="dsb")
                            nc.scalar.copy(out=dsb[:], in_=g[64:65, bh, h, :])
                            psb = psCbc.tile([HC, M], F32, tag="psCbc")
                            nc.tensor.matmul(psb[:], ones64[:], dsb[:],
                                             start=True, stop=True)
                            rec = smp.tile([HC, M], F32, tag="rec")
                            nc.vector.reciprocal(out=rec[:], in_=psb[:])
                            out1T = smp.tile([HC, M], F32R, tag="out1T")
                            nc.vector.tensor_tensor(
                                out=out1T[:], in0=g[0:HC, bh, h, :], in1=rec[:],
                                op=mybir.AluOpType.mult)
                            # kv2 = Wkv2.T @ out1T ; col layout puts k2 at
                            # partitions po..po+64, v2T at the other half
                            wkv2 = wkv2e if h % 2 == 0 else wkv2o
                            pskv = psCkv2.tile([128, M], F32, tag="psCkv2")
                            nc.tensor.matmul(pskv[:], wkv2[:], out1T[:],
                                             start=True, stop=True)
                            nc.vector.tensor_copy(out=k2sb[po:po + 64, :],
                                                  in_=pskv[po:po + 64, :])
                            vo = 64 - po
                            v2tT = smp.tile([128, M], F32R, tag="v2tT")
                            nc.vector.tensor_copy(out=v2tT[vo:vo + 64, :],
                                                  in_=pskv[vo:vo + 64, :])
                            v2aug = v2p.tile([128, 2, HC + 1], F32R, tag="v2aug")
                            nc.vector.tensor_copy(
                                out=v2aug[:, :, HC:HC + 1],
                                in_=onescol[:, 0:1].to_broadcast([128, 2, 1]))
                            for mo in range(2):
                                pst = psCtr.tile([128, HC], F32R, tag="psCtr")
                                nc.tensor.matmul(
                                    pst[:],
                                    v2tT[vo:vo + 64, mo * 128:(mo + 1) * 128],
                                    ident[vo:vo + 64, vo:vo + 64],
                                    is_transpose=True, start=True, stop=True)
                                nc.vector.tensor_copy(out=v2aug[:, mo, 0:HC], in_=pst[:])
                            v2augs.append(v2aug)

                        for h in (2 * hp, 2 * hp + 1):
                            po = (h % 2) * 64
                            v2aug = v2augs[h % 2]
                            p2 = p2p.tile([128, 2, NLOC], F32R, tag="p2")
                            for mo in range(2):
                                for ncx in range(2):
                                    ps = psC512.tile([128, 512], F32, tag="psC512")
                                    nc.tensor.matmul(
                                        ps[:],
                                        k2sb[po:po + 64, mo * 128:(mo + 1) * 128],
                                        q3T[po:po + 64, h // 2,
                                            ncx * 512:(ncx + 1) * 512],
                                        start=True, stop=True)
                                    nc.scalar.activation(
                                        p2[:, mo, ncx * 512:(ncx + 1) * 512],
                                        ps[:], EXP, scale=0.125)
                            for ncx in range(2):
                                psa = psCav2.tile([65, 512], F32, tag="psCav2")
                                for mo in range(2):
                                    nc.tensor.matmul(
                                        psa[:], v2aug[:, mo, :],
                                        p2[:, mo, ncx * 512:(ncx + 1) * 512],
                                        start=(mo == 0), stop=(mo == 1))
                                dsb2 = smp.tile([1, 512], F32R, tag="dsb2")
                                nc.scalar.copy(out=dsb2[:], in_=psa[64:65, :])
                                psb2 = psCbc.tile([HC, 512], F32, tag="psCbc")
                                nc.tensor.matmul(psb2[:], ones64[:], dsb2[:],
                                                 start=True, stop=True)
                                rec2 = smp.tile([HC, 512], F32, tag="rec2")
                                nc.vector.reciprocal(out=rec2[:], in_=psb2[:])
                                nc.vector.tensor_tensor(
                                    out=yT[po:po + 64, h // 2,
                                           ncx * 512:(ncx + 1) * 512],
                                    in0=psa[0:HC, :], in1=rec2[:],
                                    op=mybir.AluOpType.mult)

                    for t8 in range(8):
                        ps = psC512.tile([128, 512], F32, tag="psC512")
                        nc.tensor.matmul(ps[:], ones128[:], bout[:],
                                         start=True, stop=False)
                        for ko in range(KO):
                            nc.tensor.matmul(
                                ps[:], yT[:, ko, t8 * 128:(t8 + 1) * 128],
                                wout[:, ko, :],
                                start=False, stop=(ko == KO - 1))
                        osb = op.tile([128, C], F32, tag="osb")
                        nc.vector.tensor_copy(out=osb[:], in_=ps[:])
                        nc.sync.dma_start(
                            out=out_ext[b, t8 * 128:(t8 + 1) * 128, :], in_=osb[:])

    _split_excess_sync_waits(nc)
    return nc


_NC_CACHE = []


def _get_nc():
    if not _NC_CACHE:
        _NC_CACHE.append(_build())
    return _NC_CACHE[0]


def _host_prep(x, Q, kv1_w, kv1_b, kv2_w, kv2_b, q3_w, q3_b, out_w, out_b):
    x = np.asarray(x, np.float32)
    Q = np.asarray(Q, np.float32)
    kv1_w = np.asarray(kv1_w, np.float32)
    kv1_b = np.asarray(kv1_b, np.float32)
    kv2_w = np.asarray(kv2_w, np.float32)
    kv2_b = np.asarray(kv2_b, np.float32)
    q3_w = np.asarray(q3_w, np.float32)
    q3_b = np.asarray(q3_b, np.float32)
    out_w = np.asarray(out_w, np.float32)
    out_b = np.asarray(out_b, np.float32)

    kv1_blk = kv1_w.reshape(KO, 128, 2 * C).transpose(1, 0, 2)
    wk1 = np.ascontiguousarray(kv1_blk[:, :, :C])
    wv1 = np.ascontiguousarray(kv1_blk[:, :, C:])
    wq3 = np.ascontiguousarray(q3_w.reshape(KO, 128, C).transpose(1, 0, 2))
    wout = np.ascontiguousarray(out_w.reshape(KO, 128, C).transpose(1, 0, 2))
    wkv2e = np.ascontiguousarray(kv2_w)                                   # [k2|v2]
    wkv2o = np.ascontiguousarray(np.concatenate([kv2_w[:, HC:], kv2_w[:, :HC]], 1))
    qTh = np.zeros((128, KO, M), np.float32)
    for h in range(H):
        qTh[(h % 2) * 64:(h % 2) * 64 + 64, h // 2, :] = Q[h].T
    bq3 = np.ascontiguousarray(q3_b.reshape(KO, 128).T)

    bv1 = kv1_b[C:]
    delta = np.zeros(C, np.float32)
    for h in range(H):
        delta[h * HC:(h + 1) * HC] = bv1[h * HC:(h + 1) * HC] @ kv2_w[:, HC:] + kv2_b[HC:]
    bout_eff = (out_b + delta @ out_w).reshape(1, C).astype(np.float32)

    shared = {"qT": qTh, "wk1": wk1, "wv1": wv1, "wq3": wq3, "wout": wout,
              "wkv2e": wkv2e, "wkv2o": wkv2o, "bq3": bq3, "bout": bout_eff}
    in_maps = []
    for i in range(NCORES):
        xT = np.ascontiguousarray(
            x[:, i * NLOC:(i + 1) * NLOC, :].transpose(0, 2, 1))
        m = dict(shared)
        m["xT"] = xT
        in_maps.append(m)
    return in_maps


def kernel(**inputs):
    in_maps = _host_prep(**inputs)
    nc = _get_nc()
    res = run_bass_kernel_spmd(nc, in_maps, core_ids=list(range(NCORES)),
                               trace=TRACE)
    kernel.last_result = res
    out = np.concatenate([res.results[i]["out"] for i in range(NCORES)], axis=1)
    return out
